# revision 5
# baseline (speedup 1.0000x reference)
"""Trainium2 Bass kernel for nn_ConvTransduce1D (self-contained).

Computes, for x [16, 4096, 128] fp32, the CTC-style automaton forward scores
out [16, 4096, 52] of 52 tiny lexicon automata (26 single-token [c], 26
two-token [c, c+1], c = 1..26, blank = 0) over sliding windows of K=5 frames
(stride 1, pad 2).

Closed form (validated against the jax reference):
  For window w, with padded frames e_t = xp[w+t] (t = 0..4):
    d^u_t = e_t[c] - e_t[0];  d^v_t = e_t[c+1] - e_t[0]
    Du = exp(d^u), Dv = exp(d^v), Sb = sum_t e_t[0]
  Linear-space recurrence over t (per window, per lexicon column):
    H += Ru;  Ru = (Ru+1)*Du_t;  Rv = (Rv+H)*Dv_t;  G2 += Rv
  out[:, 0:26] = ln(H + Ru) + Sb;  out[:, 26:52] = ln(G2) + Sb
fp32/bf16 linear space is safe: |path scores| <= ~30.

Sharding: pure data parallel — batch 16 split as 2 per core across 8 cores.
Host prep per shard: zero-pad time dim by 2 and slice channels 0..27 (the
only channels the automata read) -> x28p [2, 4100, 28] contiguous.

Perf: recurrence planes in bf16 (DVE 2x tensor_tensor / 4x tensor_scalar);
(Ru+1)*Du is tensor_scalar(+1)+tensor_tensor (scalar_tensor_tensor is
1x-only). XDEU/XDEV exp tiles are 28-col padded so t-shifted window reads
stay 4B-aligned. Pool engine carries the H prefix chain; ACT does exp/ln
and small copies. Plane tiles rotate (bufs=4) to avoid WAR serialization.
"""

from contextlib import ExitStack

import numpy as np

import concourse.bacc as bacc
import concourse.bass as bass
import concourse.mybir as mybir
import concourse.tile as tile
from concourse.bass_utils import run_bass_kernel_spmd

F32 = mybir.dt.float32
BF16 = mybir.dt.bfloat16
A = mybir.AluOpType
AF = mybir.ActivationFunctionType

B_FULL, T, C = 16, 4096, 128
KTAPS = 5
PAD = 2
TP = T + 2 * PAD
CH = 28          # channels shipped: blank + labels 1..27
NK = 26          # lexicon entries per type
NCOL = 52        # output channels
N_CORES = 8
B_CORE = B_FULL // N_CORES


def _mkap(ap, dims, extra_offset=0):
    """Manual AP on the same tensor: keep partition dim, replace free dims."""
    part = ap.ap[0]
    return bass.AP(ap.tensor, ap.offset + extra_offset,
                   [list(part)] + [list(d) for d in dims])


def _build_core_kernel(nc, w_pp=32, b_core=B_CORE, dt_rec=BF16):
    x = nc.declare_dram_parameter("x", [b_core, TP, CH], F32, isOutput=False)
    y = nc.declare_dram_parameter("y", [b_core, T, NCOL], F32, isOutput=True)

    n_chunks = T // (128 * w_pp)
    rows = w_pp + KTAPS - 1

    with ExitStack() as ctx:
        tc = ctx.enter_context(tile.TileContext(nc))
        pool = ctx.enter_context(tc.tile_pool(name="main", bufs=2))
        rot = ctx.enter_context(tc.tile_pool(name="rot", bufs=4))

        v = nc.vector
        g = nc.gpsimd
        s = nc.scalar

        for b in range(b_core):
            for c in range(n_chunks):
                base = c * 128 * w_pp
                X3 = pool.tile([128, rows, CH], F32, tag="X3")
                # 4 partition-quarter DMAs -> parallel HWDGE queues
                for q in range(4):
                    nc.sync.dma_start(
                        out=X3[q * 32:(q + 1) * 32, :, :],
                        in_=bass.AP(x, (b * TP + base + q * 32 * w_pp) * CH,
                                    [[w_pp * CH, 32], [CH, rows], [1, CH]]))

                XD = pool.tile([128, rows, CH - 1], F32, tag="XD")
                v.tensor_tensor(XD[:], X3[:, :, 1:CH],
                                X3[:, :, 0:1].broadcast_to(
                                    [128, rows, CH - 1]), A.subtract)
                # aligned bf16 exp tiles (28-wide rows; cols 0:26 used)
                XU = pool.tile([128, rows, CH], dt_rec, tag="XU")
                XV = pool.tile([128, rows, CH], dt_rec, tag="XV")
                s.activation(XU[:, :, 0:NK], XD[:, :, 0:NK], AF.Exp)
                s.activation(XV[:, :, 0:NK], XD[:, :, 1:NK + 1], AF.Exp)

                Sb = pool.tile([128, w_pp], F32, tag="Sb")
                v.tensor_reduce(
                    Sb[:], _mkap(X3[:], [[CH, w_pp], [CH, KTAPS]]),
                    mybir.AxisListType.X, A.add)

                def Du(t):
                    return XU[:, t:t + w_pp, 0:NK]

                def Dv(t):
                    return XV[:, t:t + w_pp, 0:NK]

                def pt(tag):
                    return rot.tile([128, w_pp, NK], dt_rec, tag=tag,
                                    name=f"{tag}_t")

                # t = 0
                Ru = pt("Ru")
                v.tensor_copy(Ru[:], Du(0))
                # t = 1
                H = pt("H")
                v.tensor_copy(H[:], Ru[:])
                Rp = pt("Rp")
                v.tensor_scalar_add(Rp[:], Ru[:], 1.0)
                Ru = pt("Ru")
                v.tensor_tensor(Ru[:], Rp[:], Du(1), A.mult)
                Rv = pt("Rv")
                v.tensor_tensor(Rv[:], H[:], Dv(1), A.mult)
                G2 = pool.tile([128, w_pp, NK], dt_rec, tag="G2")
                s.activation(G2[:], Rv[:], AF.Copy)
                # t = 2..4
                for t in range(2, KTAPS):
                    Hn = pt("H")
                    g.tensor_tensor(Hn[:], H[:], Ru[:], A.add)
                    H = Hn
                    Rp = pt("Rp")
                    v.tensor_scalar_add(Rp[:], Ru[:], 1.0)
                    Run = pt("Ru")
                    v.tensor_tensor(Run[:], Rp[:], Du(t), A.mult)
                    Tt = pt("Tt")
                    v.tensor_tensor(Tt[:], Rv[:], H[:], A.add)
                    Rvn = pt("Rv")
                    v.tensor_tensor(Rvn[:], Tt[:], Dv(t), A.mult)
                    Ru, Rv = Run, Rvn
                    if t == 2:
                        g.tensor_tensor(G2[:], G2[:], Rv[:], A.add)
                    else:
                        v.tensor_tensor(G2[:], G2[:], Rv[:], A.add)

                G1 = pt("Tt")
                g.tensor_tensor(G1[:], H[:], Ru[:], A.add)

                OUT = pool.tile([128, w_pp, NCOL], F32, tag="OUT")
                s.activation(OUT[:, :, 0:NK], G1[:], AF.Ln)
                s.activation(OUT[:, :, NK:NCOL], G2[:], AF.Ln)
                # Sb add split by type half on different engines so the
                # type-1 half proceeds while Ln(G2) is still running
                sb_ap = _mkap(Sb[:], [[1, w_pp], [0, NK]])
                g.tensor_tensor(OUT[:, :, 0:NK], OUT[:, :, 0:NK], sb_ap, A.add)
                v.tensor_tensor(OUT[:, :, NK:NCOL], OUT[:, :, NK:NCOL],
                                sb_ap, A.add)

                # 4 partition-quarter DMAs -> parallel HWDGE queues
                for q in range(4):
                    nc.sync.dma_start(
                        out=bass.AP(y, b * T * NCOL + (base + q * 32 * w_pp)
                                    * NCOL,
                                    [[w_pp * NCOL, 32], [NCOL, w_pp],
                                     [1, NCOL]]),
                        in_=OUT[q * 32:(q + 1) * 32, :, :])
    return nc


_NC_CACHE = {}


def _get_nc():
    if "nc" not in _NC_CACHE:
        nc = bacc.Bacc()
        _build_core_kernel(nc)
        nc.compile()
        _NC_CACHE["nc"] = nc
    return _NC_CACHE["nc"]


def _prep_shard(x_shard):
    """[B_CORE, T, C] -> zero-padded, channel-sliced [B_CORE, TP, CH]."""
    out = np.zeros((x_shard.shape[0], TP, CH), np.float32)
    out[:, PAD:PAD + T, :] = x_shard[:, :, 0:CH]
    return out


def _run(x, trace=False, **kw):
    x = np.asarray(x, dtype=np.float32)
    assert x.shape == (B_FULL, T, C), x.shape
    nc = _get_nc()
    in_maps = [{"x": _prep_shard(x[i * B_CORE:(i + 1) * B_CORE])}
               for i in range(N_CORES)]
    res = run_bass_kernel_spmd(nc, in_maps, list(range(N_CORES)),
                               trace=trace, **kw)
    out = np.concatenate([res.results[i]["y"] for i in range(N_CORES)], axis=0)
    return np.ascontiguousarray(out.astype(np.float32)), res


def kernel(x):
    out, _ = _run(x, trace=False)
    return out


# revision 8
# speedup vs baseline: 1.1110x; 1.1110x over previous
"""Trainium2 Bass kernel for nn_ConvTransduce1D (self-contained).

Computes, for x [16, 4096, 128] fp32, the CTC-style automaton forward scores
out [16, 4096, 52] of 52 tiny lexicon automata (26 single-token [c], 26
two-token [c, c+1], c = 1..26, blank = 0) over sliding windows of K=5 frames
(stride 1, pad 2).

Closed form (validated against the jax reference):
  For window w, with padded frames e_t = xp[w+t] (t = 0..4):
    d^u_t = e_t[c] - e_t[0];  d^v_t = e_t[c+1] - e_t[0]
    Du = exp(d^u), Dv = exp(d^v), Sb = sum_t e_t[0]
  Linear-space recurrence over t (per window, per lexicon column):
    H += Ru;  Ru = (Ru+1)*Du_t;  Rv = (Rv+H)*Dv_t;  G2 += Rv
  out[:, 0:26] = ln(H + Ru) + Sb;  out[:, 26:52] = ln(G2) + Sb
fp32/bf16 linear space is safe: |path scores| <= ~30.

Sharding: pure data parallel — batch 16 split as 2 per core across 8 cores.
Host prep per shard: zero-pad time dim by 2 and slice channels 0..27 (the
only channels the automata read) -> x28p [2, 4100, 28] contiguous.

Perf: recurrence planes in bf16 (DVE 2x tensor_tensor / 4x tensor_scalar);
(Ru+1)*Du is tensor_scalar(+1)+tensor_tensor (scalar_tensor_tensor is
1x-only). XDEU/XDEV exp tiles are 28-col padded so t-shifted window reads
stay 4B-aligned. Pool engine carries the H prefix chain; ACT does exp/ln
and small copies. Plane tiles rotate (bufs=4) to avoid WAR serialization.
"""

from contextlib import ExitStack

import numpy as np

import concourse.bacc as bacc
import concourse.bass as bass
import concourse.mybir as mybir
import concourse.tile as tile
from concourse.bass_utils import run_bass_kernel_spmd

F32 = mybir.dt.float32
BF16 = mybir.dt.bfloat16
A = mybir.AluOpType
AF = mybir.ActivationFunctionType

B_FULL, T, C = 16, 4096, 128
KTAPS = 5
PAD = 2
TP = T + 2 * PAD
CH = 28          # channels shipped: blank + labels 1..27
NK = 26          # lexicon entries per type
NCOL = 52        # output channels
N_CORES = 8
B_CORE = B_FULL // N_CORES


def _mkap(ap, dims, extra_offset=0):
    """Manual AP on the same tensor: keep partition dim, replace free dims."""
    part = ap.ap[0]
    return bass.AP(ap.tensor, ap.offset + extra_offset,
                   [list(part)] + [list(d) for d in dims])


def _build_core_kernel(nc, w_pp=32, b_core=B_CORE, dt_rec=BF16):
    x = nc.declare_dram_parameter("x", [b_core, TP, CH], F32, isOutput=False)
    y = nc.declare_dram_parameter("y", [b_core, T, NCOL], F32, isOutput=True)

    n_chunks = T // (128 * w_pp)
    rows = w_pp + KTAPS - 1

    with ExitStack() as ctx:
        tc = ctx.enter_context(tile.TileContext(nc))
        pool = ctx.enter_context(tc.tile_pool(name="main", bufs=2))
        rot = ctx.enter_context(tc.tile_pool(name="rot", bufs=4))

        v = nc.vector
        g = nc.gpsimd
        s = nc.scalar

        for b in range(b_core):
            for c in range(n_chunks):
                base = c * 128 * w_pp
                X3 = pool.tile([128, rows, CH], F32, tag="X3")
                nc.sync.dma_start(
                    out=X3[:],
                    in_=bass.AP(x, (b * TP + base) * CH,
                                [[w_pp * CH, 128], [CH, rows], [1, CH]]))

                XD = pool.tile([128, rows, CH - 1], F32, tag="XD")
                v.tensor_tensor(XD[:], X3[:, :, 1:CH],
                                X3[:, :, 0:1].broadcast_to(
                                    [128, rows, CH - 1]), A.subtract)
                # aligned bf16 exp tiles (28-wide rows; cols 0:26 used)
                XU = pool.tile([128, rows, CH], dt_rec, tag="XU")
                XV = pool.tile([128, rows, CH], dt_rec, tag="XV")
                s.activation(XU[:, :, 0:NK], XD[:, :, 0:NK], AF.Exp)
                s.activation(XV[:, :, 0:NK], XD[:, :, 1:NK + 1], AF.Exp)

                Sb = pool.tile([128, w_pp], F32, tag="Sb")
                v.tensor_reduce(
                    Sb[:], _mkap(X3[:], [[CH, w_pp], [CH, KTAPS]]),
                    mybir.AxisListType.X, A.add)

                def Du(t):
                    return XU[:, t:t + w_pp, 0:NK]

                def Dv(t):
                    return XV[:, t:t + w_pp, 0:NK]

                def pt(tag):
                    return rot.tile([128, w_pp, NK], dt_rec, tag=tag,
                                    name=f"{tag}_t")

                # t = 0
                Ru = pt("Ru")
                v.tensor_copy(Ru[:], Du(0))
                # t = 1
                H = pt("H")
                v.tensor_copy(H[:], Ru[:])
                Rp = pt("Rp")
                v.tensor_scalar_add(Rp[:], Ru[:], 1.0)
                Ru = pt("Ru")
                v.tensor_tensor(Ru[:], Rp[:], Du(1), A.mult)
                Rv = pt("Rv")
                v.tensor_tensor(Rv[:], H[:], Dv(1), A.mult)
                G2 = pool.tile([128, w_pp, NK], dt_rec, tag="G2")
                s.activation(G2[:], Rv[:], AF.Copy)
                # t = 2..4
                for t in range(2, KTAPS):
                    Hn = pt("H")
                    g.tensor_tensor(Hn[:], H[:], Ru[:], A.add)
                    H = Hn
                    Rp = pt("Rp")
                    v.tensor_scalar_add(Rp[:], Ru[:], 1.0)
                    Run = pt("Ru")
                    v.tensor_tensor(Run[:], Rp[:], Du(t), A.mult)
                    Tt = pt("Tt")
                    v.tensor_tensor(Tt[:], Rv[:], H[:], A.add)
                    Rvn = pt("Rv")
                    v.tensor_tensor(Rvn[:], Tt[:], Dv(t), A.mult)
                    Ru, Rv = Run, Rvn
                    if t in (2, 3):
                        g.tensor_tensor(G2[:], G2[:], Rv[:], A.add)
                    else:
                        v.tensor_tensor(G2[:], G2[:], Rv[:], A.add)

                G1 = pt("Tt")
                v.tensor_tensor(G1[:], H[:], Ru[:], A.add)

                OUT = pool.tile([128, w_pp, NCOL], F32, tag="OUT")
                s.activation(OUT[:, :, 0:NK], G1[:], AF.Ln)
                s.activation(OUT[:, :, NK:NCOL], G2[:], AF.Ln)
                # Sb add split by type half so the type-1 half (and its
                # DMA) proceeds while Ln(G2) is still running
                sb_ap = _mkap(Sb[:], [[1, w_pp], [0, NK]])
                v.tensor_tensor(OUT[:, :, 0:NK], OUT[:, :, 0:NK], sb_ap, A.add)
                v.tensor_tensor(OUT[:, :, NK:NCOL], OUT[:, :, NK:NCOL],
                                sb_ap, A.add)

                nc.sync.dma_start(
                    out=bass.AP(y, b * T * NCOL + base * NCOL,
                                [[w_pp * NCOL, 128], [NCOL, w_pp], [1, NCOL]]),
                    in_=OUT[:])
    return nc


_NC_CACHE = {}


def _get_nc():
    if "nc" not in _NC_CACHE:
        nc = bacc.Bacc()
        _build_core_kernel(nc)
        nc.compile()
        _NC_CACHE["nc"] = nc
    return _NC_CACHE["nc"]


def _prep_shard(x_shard):
    """[B_CORE, T, C] -> zero-padded, channel-sliced [B_CORE, TP, CH]."""
    out = np.zeros((x_shard.shape[0], TP, CH), np.float32)
    out[:, PAD:PAD + T, :] = x_shard[:, :, 0:CH]
    return out


def _run(x, trace=False, **kw):
    x = np.asarray(x, dtype=np.float32)
    assert x.shape == (B_FULL, T, C), x.shape
    nc = _get_nc()
    in_maps = [{"x": _prep_shard(x[i * B_CORE:(i + 1) * B_CORE])}
               for i in range(N_CORES)]
    res = run_bass_kernel_spmd(nc, in_maps, list(range(N_CORES)),
                               trace=trace, **kw)
    out = np.concatenate([res.results[i]["y"] for i in range(N_CORES)], axis=0)
    return np.ascontiguousarray(out.astype(np.float32)), res


def kernel(x):
    out, _ = _run(x, trace=False)
    return out
